# revision 1
# baseline (speedup 1.0000x reference)
"""HGNN layer kernel for 8 TRN2 NeuronCores (Bass/Tile, SPMD row-sharded).

Math (reference):
    dv = H.sum(1); de = H.sum(0)
    Xs = X * dv^-1/2
    M  = H^T @ Xs            [E, F]
    M  = M * de^-1
    Xn = (H @ M) * dv^-1/2   [N, F]
    out = Xn @ W^T + b

Distribution: rows of X/H sharded over 8 cores (N=8192 -> 1024 rows/core).
GEMM1 (H^T @ Xs) is a local partial GEMM; the [E, F] partial plus the
partial column-sum row `de` are fused into ONE AllReduce of [F+1, E].
Everything after that is row-parallel.

Layout trick: GEMM1 is computed transposed (M^T = Xs^T-as-stationary, H
moving) so the AllReduce buffer is [F+1, E] with partition=f. Post-AR,
M'^T chunks [fi,128e] serve as matmul *stationary* operands against the
moving W^T, which lands Mw in [e(part), fo] layout directly -- no on-chip
transposes anywhere (H^T comes pre-transposed from the host shard prep).
"""

import os
import sys
import types

import numpy as np


def _ensure_axon_hooks_module():
    """bass_utils imports antenv.axon_hooks when tracing; some images
    lack it. Provide a stub (and try to wire the real ctypes hook) so
    trace paths degrade gracefully instead of crashing."""
    try:
        import antenv.axon_hooks  # noqa: F401
        return
    except ImportError:
        pass
    try:
        import antenv
    except ImportError:
        return
    mod = types.ModuleType("antenv.axon_hooks")
    state = {"hook": None}
    mod.get_axon_ntff_profile_hook = lambda: state["hook"]
    mod.set_axon_ntff_profile_hook = lambda h: state.__setitem__("hook", h)
    sys.modules["antenv.axon_hooks"] = mod
    antenv.axon_hooks = mod
    try:
        from trn_agent_boot.trn_boot import _ntff_profile_via_ctypes
        hook = _ntff_profile_via_ctypes("/opt/axon/libaxon_pjrt.so")
        if hook is not None:
            state["hook"] = hook
    except Exception:
        pass


_ensure_axon_hooks_module()

N, E, F = 8192, 1024, 256
P = 128
NC_COUNT = 8
NL = N // NC_COUNT          # 1024 rows per core
NT = NL // P                # 8 row tiles per core
ET = E // P                 # 8 e-chunks
FI = F // P                 # 2 fi-chunks

# matmul compute dtype: "f32r" (full-rate, ~tf32 precision), "f32" (1/4 rate,
# full precision). H/ones stay exact in either mode.
MM_DTYPE = os.environ.get("HGNN_MM_DTYPE", "f32r")

_cache = {}


def _build():
    from concourse import bacc, bass, tile, mybir

    f32 = mybir.dt.float32

    nc = bacc.Bacc("TRN2", target_bir_lowering=False, debug=False,
                   num_devices=NC_COUNT)

    X_d = nc.dram_tensor("X", [NL, F], f32, kind="ExternalInput")
    H_d = nc.dram_tensor("H", [NL, E], f32, kind="ExternalInput")
    HT_d = nc.dram_tensor("HT", [E, NL], f32, kind="ExternalInput")
    WT_d = nc.dram_tensor("WT", [F, F], f32, kind="ExternalInput")
    B_d = nc.dram_tensor("bias", [P, F], f32, kind="ExternalInput")
    ONES_d = nc.dram_tensor("ones", [P, 1], f32, kind="ExternalInput")
    out_d = nc.dram_tensor("out", [NL, F], f32, kind="ExternalOutput")

    if MM_DTYPE == "f32r":
        R = mybir.dt.float32r

        def rc(ap):
            return ap.bitcast(R)
    else:
        R = f32

        def rc(ap):
            return ap

    with tile.TileContext(nc) as tc:
        with (
            tc.tile_pool(name="const", bufs=1) as constp,
            tc.tile_pool(name="hp", bufs=1) as hp,
            tc.tile_pool(name="htp", bufs=1) as htp,
            tc.tile_pool(name="xp", bufs=1) as xp,
            tc.tile_pool(name="sp", bufs=1) as sp,
            tc.tile_pool(name="mtout", bufs=4) as mtoutp,
            tc.tile_pool(name="mwp", bufs=1) as mwp,
            tc.tile_pool(name="outp", bufs=3) as outp,
            tc.tile_pool(name="ps_mt", bufs=2, space="PSUM") as ps_mt,
            tc.tile_pool(name="ps_de", bufs=2, space="PSUM") as ps_de,
            tc.tile_pool(name="ps_b", bufs=3, space="PSUM") as ps_b,
            tc.tile_pool(name="dram", bufs=1, space="DRAM") as dramp,
        ):
            # ---- ones first (gates the de matmuls at the head of the PE
            # stream), then H on the sync queue while X/consts go via gpsimd.
            ones = constp.tile([P, 1], R)
            nc.sync.dma_start(ones[:], rc(ONES_d[:, :]))

            h = []
            for i in range(NT):
                hi = hp.tile([P, E], R, name=f"h{i}")
                nc.sync.dma_start(hi[:], rc(H_d[i * P:(i + 1) * P, :]))
                h.append(hi)

            x = []
            for i in range(NT):
                xi = xp.tile([P, F], f32, name=f"x{i}")
                nc.gpsimd.dma_start(xi[:], X_d[i * P:(i + 1) * P, :])
                x.append(xi)

            wt = []
            for c in range(FI):
                wtc = constp.tile([P, F], R, name=f"wt{c}")
                nc.gpsimd.dma_start(wtc[:], rc(WT_d[c * P:(c + 1) * P, :]))
                wt.append(wtc)
            bias = constp.tile([P, F], f32)
            nc.gpsimd.dma_start(bias[:], B_d[:, :])

            # dv chain (per tile): DVE rowsum -> DVE recip -> ACT sqrt -> DVE mul
            xs, dvis = [], []
            for i in range(NT):
                dv = sp.tile([P, 1], f32, name=f"dv{i}")
                nc.vector.tensor_reduce(dv[:], h[i][:].bitcast(f32),
                                        mybir.AxisListType.X,
                                        mybir.AluOpType.add)
                dvr = sp.tile([P, 1], f32, name=f"dvr{i}")
                nc.vector.reciprocal(dvr[:], dv[:])
                dvi = sp.tile([P, 1], f32, name=f"dvis{i}")
                nc.scalar.sqrt(dvi[:], dvr[:])
                dvis.append(dvi)

                xsi = xp.tile([P, F], R, name=f"xs{i}")
                nc.vector.tensor_scalar_mul(xsi[:], x[i][:], dvi[:])
                xs.append(xsi)

            # ---- collective bounce buffers ----
            cc_in = dramp.tile([F + 1, E], f32, name="cc_in")
            cc_out = dramp.tile([F + 1, E], f32, name="cc_out",
                                addr_space="Shared")

            # ---- de row first: de[e] = sum_n H[n, e] (needs only H, so the
            # PE computes it while the dv/xs chain is still running) ----
            EH = 512  # moving free-dim per matmul
            for eh in range(E // EH):
                de_ps = ps_de.tile([1, EH], f32, name="de_ps")
                for i in range(NT):
                    nc.tensor.matmul(
                        de_ps[:], ones[:],
                        h[i][:, eh * EH:(eh + 1) * EH],
                        start=(i == 0), stop=(i == NT - 1),
                    )
                de_sb = mtoutp.tile([1, EH], f32, name="de_sb")
                nc.scalar.copy(de_sb[:], de_ps[:])
                nc.sync.dma_start(cc_in[F:F + 1, eh * EH:(eh + 1) * EH],
                                  de_sb[:])

            # ---- GEMM1: M^T[f, e] = sum_n Xs[n, f] * H[n, e] ----
            for jf in range(FI):
                for eh in range(E // EH):
                    mt_ps = ps_mt.tile([P, EH], f32, name="mt_ps")
                    for i in range(NT):
                        nc.tensor.matmul(
                            mt_ps[:],
                            xs[i][:, jf * P:(jf + 1) * P],
                            h[i][:, eh * EH:(eh + 1) * EH],
                            start=(i == 0), stop=(i == NT - 1),
                        )
                    mt_sb = mtoutp.tile([P, EH], f32, name="mt_sb")
                    nc.vector.tensor_copy(mt_sb[:], mt_ps[:])
                    nc.sync.dma_start(
                        cc_in[jf * P:(jf + 1) * P, eh * EH:(eh + 1) * EH],
                        mt_sb[:])

            # ---- AllReduce of [M^T | de] over all 8 cores ----
            nc.gpsimd.collective_compute(
                "AllReduce",
                mybir.AluOpType.add,
                replica_groups=[list(range(NC_COUNT))],
                ins=[cc_in[:].opt()],
                outs=[cc_out[:].opt()],
            )

            # ---- H^T tiles (host-pretransposed); overlap with AllReduce ----
            ht = []
            for j in range(ET):
                htj = htp.tile([P, NL], R, name=f"ht{j}")
                nc.gpsimd.dma_start(htj[:], rc(HT_d[j * P:(j + 1) * P, :]))
                ht.append(htj)

            # ---- read back: M'^T fi-chunks + de (reshaped to [128, 8]) ----
            mtin = []
            for c in range(FI):
                mc = mwp.tile([P, E], R, name=f"mtin{c}")
                nc.sync.dma_start(mc[:], rc(cc_out[c * P:(c + 1) * P, :]))
                mtin.append(mc)
            de_sb2 = sp.tile([P, ET], f32)
            nc.sync.dma_start(
                de_sb2[:],
                cc_out[F:F + 1, :].rearrange("o (c p) -> (o p) c", p=P))
            de_inv = sp.tile([P, ET], f32)
            nc.vector.reciprocal(de_inv[:], de_sb2[:])

            # ---- GEMM-W: Mw[e, fo] = sum_fi M'[e, fi] W^T[fi, fo]; x de^-1 ----
            mw = []
            for j in range(ET):
                mw_ps = ps_b.tile([P, F], f32, name="mw_ps", tag="ps_post")
                for c in range(FI):
                    nc.tensor.matmul(
                        mw_ps[:],
                        mtin[c][:, j * P:(j + 1) * P],
                        wt[c][:],
                        start=(c == 0), stop=(c == FI - 1),
                    )
                mwj = mwp.tile([P, F], R, name=f"mw{j}")
                nc.vector.tensor_scalar_mul(mwj[:], mw_ps[:],
                                            de_inv[:, j:j + 1])
                mw.append(mwj)

            # ---- GEMM2: out[n, fo] = (sum_e H^T[e,n] Mw[e,fo]) * dv^-1/2 + b ----
            for jn in range(NT):
                o_ps = ps_b.tile([P, F], f32, name="o_ps", tag="ps_post")
                for j in range(ET):
                    nc.tensor.matmul(
                        o_ps[:],
                        ht[j][:, jn * P:(jn + 1) * P],
                        mw[j][:],
                        start=(j == 0), stop=(j == ET - 1),
                    )
                ot = outp.tile([P, F], f32, name="ot")
                nc.vector.scalar_tensor_tensor(
                    ot[:], o_ps[:], dvis[jn][:], bias[:],
                    op0=mybir.AluOpType.mult, op1=mybir.AluOpType.add)
                nc.sync.dma_start(out_d[jn * P:(jn + 1) * P, :], ot[:])

    nc.compile()
    return nc


def _get_nc():
    if "nc" not in _cache:
        _cache["nc"] = _build()
    return _cache["nc"]


def kernel(X, H, W, b):
    from concourse import bass_utils

    nc = _get_nc()

    X = np.asarray(X, dtype=np.float32)
    H = np.asarray(H, dtype=np.float32)
    W = np.asarray(W, dtype=np.float32)
    b = np.asarray(b, dtype=np.float32)

    WT = np.ascontiguousarray(W.T)
    bias = np.ascontiguousarray(np.tile(b[None, :], (P, 1)))
    ones_col = np.ones((P, 1), dtype=np.float32)

    in_maps = []
    for c in range(NC_COUNT):
        sl = slice(c * NL, (c + 1) * NL)
        Hc = np.ascontiguousarray(H[sl])
        in_maps.append({
            "X": np.ascontiguousarray(X[sl]),
            "H": Hc,
            "HT": np.ascontiguousarray(Hc.T),
            "WT": WT,
            "bias": bias,
            "ones": ones_col,
        })

    res = bass_utils.run_bass_kernel_spmd(
        nc, in_maps, core_ids=list(range(NC_COUNT)),
        trace=bool(int(os.environ.get("HGNN_TRACE", "0"))),
    )
    _cache["last_result"] = res
    out = np.concatenate([res.results[c]["out"] for c in range(NC_COUNT)],
                         axis=0)
    return out

